# revision 4
# baseline (speedup 1.0000x reference)
"""Trainium2 Bass kernel for nn_EuclideanEmbedding (vq_codebook).

reference:
    distances = cdist(x, p)                      # (8192, 512)
    r1 = mean_j min_i distances[i, j]            # scalar
    r2 = mean_i min_j distances[i, j]            # scalar

Strategy (8 NeuronCores):
  - Shard x along batch: 1024 rows/core; replicate the (512, 64) codebook.
  - Per core, squared distances via ONE augmented matmul per 128-row tile:
      D2 = [x, ||x||^2, 1] @ [-2 p, 1, ||p||^2]^T     (K = 66)
    Tiles are processed in pairs ([128, 2, 512] PSUM supertiles) to
    amortize per-instruction overheads.
  - ScalarE: sqrt(D2) PSUM->SBUF (one op per pair); DMA streams the
    distance shard out (contiguous 256KB blocks).
  - VectorE r2 partials: per-row min over the codebook on PSUM D2.
  - VectorE r1 partials: a SECOND set of matmuls emits the transposed
    layout D2T[j, batch] (the tensor engine has spare capacity), so the
    min-over-batch is also a free-dim reduce - no elementwise chain, no
    on-chip transposes. Only a [128, 4] vector leaves each core.
  - Host combines the per-core partial minima (the cross-device min /
    mean all-reduce of the sharding hint) while unsharding.
"""

import numpy as np

import concourse.bacc as bacc
import concourse.tile as tile
from concourse import mybir
from concourse.bass_utils import run_bass_kernel_spmd

BATCH = 8192
NV = 512          # codebook vectors
D = 64            # latent dim
NCORES = 8
BPC = BATCH // NCORES   # 1024 batch rows per core
P = 128                 # partitions
NTILES = BPC // P       # 8 tiles of 128 batch rows per core
NPAIRS = NTILES // 2    # 4 supertiles of [128, 2, 512]
NJC = NV // P           # 4 codebook chunks of 128
KAUG = D + 2            # augmented contraction dim

_CACHE = {}


def _make_nc():
    return bacc.Bacc(
        "TRN2",
        target_bir_lowering=False,
        debug=False,
        enable_asserts=False,
        num_devices=NCORES,
    )


def _declare_io(nc):
    f32 = mybir.dt.float32
    return {
        "xaugT": nc.dram_tensor("xaugT", [KAUG, BPC], f32, kind="ExternalInput").ap(),
        "paugT": nc.dram_tensor("paugT", [KAUG, NV], f32, kind="ExternalInput").ap(),
        "dist": nc.dram_tensor("dist", [BPC, NV], f32, kind="ExternalOutput").ap(),
        "colfin": nc.dram_tensor("colfin", [P, NJC], f32, kind="ExternalOutput").ap(),
        "rowmin": nc.dram_tensor("rowmin", [P, NTILES], f32, kind="ExternalOutput").ap(),
    }


def _emit_body(nc, io, singles, dists, psums, apsums, xaugT_sb, paugT_sb):
    """One full pass over the core's 1024 x 512 distance block."""
    f32 = mybir.dt.float32
    mn = mybir.AluOpType.min
    rowmin_sb = singles.tile([P, NTILES], f32)
    colfin_sb = singles.tile([P, NJC], f32)

    for q in range(NPAIRS):
        psum_pair = psums.tile([P, 2, NV], f32)
        for h in range(2):
            t = 2 * q + h
            nc.tensor.matmul(
                psum_pair[:, h, :],
                xaugT_sb[:, t * P:(t + 1) * P],
                paugT_sb[:],
                start=True,
                stop=True,
            )
        dist_pair = dists.tile([P, 2, NV], f32)
        nc.scalar.sqrt(dist_pair[:], psum_pair[:])
        nc.sync.dma_start(
            out=io["dist"][2 * q * P:(2 * q + 2) * P, :].rearrange(
                "(b p) j -> p b j", b=2),
            in_=dist_pair[:],
        )
        # r2 partials: min over the 512 codebook entries per row (D2 domain)
        nc.vector.tensor_reduce(
            rowmin_sb[:, 2 * q:2 * q + 2],
            psum_pair[:],
            axis=mybir.AxisListType.X,
            op=mn,
        )

    # r1 partials via transposed-layout matmuls: D2T[j, batch]
    for jc in range(NJC):
        apsum = apsums.tile([P, 2, NV], f32)   # [128 j, 1024 batch]
        for h in range(2):
            nc.tensor.matmul(
                apsum[:, h, :],
                paugT_sb[:, jc * P:(jc + 1) * P],
                xaugT_sb[:, h * NV:(h + 1) * NV],
                start=True,
                stop=True,
            )
        nc.vector.tensor_reduce(
            colfin_sb[:, jc:jc + 1],
            apsum[:],
            axis=mybir.AxisListType.XY,
            op=mn,
        )

    nc.sync.dma_start(out=io["colfin"][:], in_=colfin_sb[:])
    nc.sync.dma_start(out=io["rowmin"][:], in_=rowmin_sb[:])


def _build_program(outer_loop=None, inner_unroll=1):
    """outer_loop=None -> single-pass production program.
    outer_loop=K -> For_i hardware loop with inner_unroll python-unrolled
    passes per iteration (timing amplification)."""
    f32 = mybir.dt.float32
    nc = _make_nc()
    io = _declare_io(nc)

    with tile.TileContext(nc) as tc:
        with (
            tc.tile_pool(name="consts", bufs=1) as consts,
            tc.tile_pool(name="singles", bufs=2) as singles,
            tc.tile_pool(name="dists", bufs=3) as dists,
            tc.tile_pool(name="psums", bufs=2, space="PSUM") as psums,
            tc.tile_pool(name="apsums", bufs=2, space="PSUM") as apsums,
        ):
            xaugT_sb = consts.tile([KAUG, BPC], f32)
            paugT_sb = consts.tile([KAUG, NV], f32)
            nc.sync.dma_start(out=xaugT_sb[:], in_=io["xaugT"][:])
            nc.sync.dma_start(out=paugT_sb[:], in_=io["paugT"][:])

            if outer_loop is None:
                _emit_body(nc, io, singles, dists, psums, apsums,
                           xaugT_sb, paugT_sb)
            else:
                with tc.For_i(0, outer_loop, 1):
                    for _ in range(inner_unroll):
                        _emit_body(nc, io, singles, dists, psums, apsums,
                                   xaugT_sb, paugT_sb)

    nc.compile()
    return nc


def _get_program():
    if "nc" not in _CACHE:
        _CACHE["nc"] = _build_program()
    return _CACHE["nc"]


def _prep_inputs(x, p):
    x = np.ascontiguousarray(np.asarray(x, dtype=np.float32))
    p = np.ascontiguousarray(np.asarray(p, dtype=np.float32))
    xsq = np.einsum("id,id->i", x.astype(np.float64), x.astype(np.float64))
    psq = np.einsum("jd,jd->j", p.astype(np.float64), p.astype(np.float64))
    xaugT = np.empty((KAUG, BATCH), np.float32)
    xaugT[:D] = x.T
    xaugT[D] = xsq.astype(np.float32)
    xaugT[D + 1] = 1.0
    paugT = np.empty((KAUG, NV), np.float32)
    paugT[:D] = -2.0 * p.T
    paugT[D] = 1.0
    paugT[D + 1] = psq.astype(np.float32)
    in_maps = []
    for c in range(NCORES):
        in_maps.append({
            "xaugT": np.ascontiguousarray(xaugT[:, c * BPC:(c + 1) * BPC]),
            "paugT": paugT,
        })
    return in_maps


def _run(x, p, trace=False, nc=None, **kwargs):
    if nc is None:
        nc = _get_program()
    in_maps = _prep_inputs(x, p)
    return run_bass_kernel_spmd(
        nc, in_maps, core_ids=list(range(NCORES)), trace=trace, **kwargs
    )


def _assemble(results):
    dist_full = np.empty((BATCH, NV), np.float32)
    colfins = np.empty((NCORES, P, NJC), np.float32)
    rowmins = np.empty((NCORES, P, NTILES), np.float32)
    for c in range(NCORES):
        r = results[c]
        dist_full[c * BPC:(c + 1) * BPC] = r["dist"]
        colfins[c] = r["colfin"]
        rowmins[c] = r["rowmin"]
    # cross-core all-reduce-min over batch (D2 domain), then codebook mean
    r1 = np.float32(np.mean(np.sqrt(colfins.min(axis=0).astype(np.float64))))
    # per-sample min is already complete locally (D2 domain); batch mean
    r2 = np.float32(np.mean(np.sqrt(rowmins.astype(np.float64))))
    return dist_full, r1, r2


def kernel(x, trainable_p):
    res = _run(x, trainable_p)
    return _assemble(res.results)


# revision 7
# speedup vs baseline: 2.5488x; 2.5488x over previous
"""Trainium2 Bass kernel for nn_EuclideanEmbedding (vq_codebook).

reference:
    distances = cdist(x, p)                      # (8192, 512)
    r1 = mean_j min_i distances[i, j]            # scalar
    r2 = mean_i min_j distances[i, j]            # scalar

Strategy (8 NeuronCores):
  - Shard x along batch: 1024 rows/core; replicate the (512, 64) codebook.
  - Per core, squared distances via ONE augmented matmul per 128-row tile:
      D2 = [x, ||x||^2, 1] @ [-2 p, 1, ||p||^2]^T     (K = 66)
    Tiles are processed in pairs ([128, 2, 512] PSUM supertiles) to
    amortize per-instruction overheads.
  - ScalarE: sqrt(D2) PSUM->SBUF (one op per pair); DMA streams the
    distance shard out (contiguous 256KB blocks).
  - VectorE r2 partials: per-row min over the codebook on PSUM D2.
  - VectorE r1 partials: a SECOND set of matmuls emits the transposed
    layout D2T[j, batch] (the tensor engine has spare capacity), so the
    min-over-batch is also a free-dim reduce - no elementwise chain, no
    on-chip transposes. Only a [128, 4] vector leaves each core.
  - Host combines the per-core partial minima (the cross-device min /
    mean all-reduce of the sharding hint) while unsharding.
"""

import numpy as np

import concourse.bacc as bacc
import concourse.tile as tile
from concourse import mybir
from concourse.bass_utils import run_bass_kernel_spmd

BATCH = 8192
NV = 512          # codebook vectors
D = 64            # latent dim
NCORES = 8
BPC = BATCH // NCORES   # 1024 batch rows per core
P = 128                 # partitions
NTILES = BPC // P       # 8 tiles of 128 batch rows per core
NPAIRS = NTILES // 2    # 4 supertiles of [128, 2, 512]
NJC = NV // P           # 4 codebook chunks of 128
KAUG = D + 2            # augmented contraction dim

_CACHE = {}


def _make_nc():
    return bacc.Bacc(
        "TRN2",
        target_bir_lowering=False,
        debug=False,
        enable_asserts=False,
        num_devices=NCORES,
    )


def _declare_io(nc):
    f32 = mybir.dt.float32
    return {
        "xaugT": nc.dram_tensor("xaugT", [KAUG, BPC], mybir.dt.float32r,
                                kind="ExternalInput").ap(),
        "paugT": nc.dram_tensor("paugT", [KAUG, NV], mybir.dt.float32r,
                                kind="ExternalInput").ap(),
        "dist": nc.dram_tensor("dist", [BPC, NV], f32, kind="ExternalOutput").ap(),
        "colfin": nc.dram_tensor("colfin", [P, NJC], f32, kind="ExternalOutput").ap(),
        "rowmin": nc.dram_tensor("rowmin", [P, NTILES], f32, kind="ExternalOutput").ap(),
    }


def _emit_body(nc, io, singles, dists, psums, apsums, xaugT_sb, paugT_sb):
    """One full pass over the core's 1024 x 512 distance block."""
    f32 = mybir.dt.float32
    f32r = mybir.dt.float32r
    mn = mybir.AluOpType.min
    rowmin_sb = singles.tile([P, NTILES], f32)
    colfin_sb = singles.tile([P, NJC], f32)
    # float32r streams 1 row/cycle on the PE (vs 4 for plain fp32)
    xaugT_r = xaugT_sb[:]
    paugT_r = paugT_sb[:]

    for q in range(NPAIRS):
        psum_pair = psums.tile([P, 2, NV], f32)
        for h in range(2):
            t = 2 * q + h
            nc.tensor.matmul(
                psum_pair[:, h, :],
                xaugT_r[:, t * P:(t + 1) * P],
                paugT_r[:],
                start=True,
                stop=True,
            )
        dist_pair = dists.tile([P, 2, NV], f32)
        nc.scalar.sqrt(dist_pair[:], psum_pair[:])
        nc.sync.dma_start(
            out=io["dist"][2 * q * P:(2 * q + 2) * P, :].rearrange(
                "(b p) j -> p b j", b=2),
            in_=dist_pair[:],
        )
        # r2 partials: min over the 512 codebook entries per row (D2 domain)
        nc.vector.tensor_reduce(
            rowmin_sb[:, 2 * q:2 * q + 2],
            psum_pair[:],
            axis=mybir.AxisListType.X,
            op=mn,
        )

    # r1 partials via transposed-layout matmuls: D2T[j, batch]
    for jc in range(NJC):
        apsum = apsums.tile([P, 2, NV], f32)   # [128 j, 1024 batch]
        for h in range(2):
            nc.tensor.matmul(
                apsum[:, h, :],
                paugT_r[:, jc * P:(jc + 1) * P],
                xaugT_r[:, h * NV:(h + 1) * NV],
                start=True,
                stop=True,
            )
        nc.vector.tensor_reduce(
            colfin_sb[:, jc:jc + 1],
            apsum[:],
            axis=mybir.AxisListType.XY,
            op=mn,
        )

    nc.sync.dma_start(out=io["colfin"][:], in_=colfin_sb[:])
    nc.sync.dma_start(out=io["rowmin"][:], in_=rowmin_sb[:])


def _build_program(outer_loop=None, inner_unroll=1):
    """outer_loop=None -> single-pass production program.
    outer_loop=K -> For_i hardware loop with inner_unroll python-unrolled
    passes per iteration (timing amplification)."""
    f32 = mybir.dt.float32
    nc = _make_nc()
    io = _declare_io(nc)

    with tile.TileContext(nc) as tc:
        with (
            tc.tile_pool(name="consts", bufs=1) as consts,
            tc.tile_pool(name="singles", bufs=2) as singles,
            tc.tile_pool(name="dists", bufs=3) as dists,
            tc.tile_pool(name="psums", bufs=2, space="PSUM") as psums,
            tc.tile_pool(name="apsums", bufs=2, space="PSUM") as apsums,
        ):
            xaugT_sb = consts.tile([KAUG, BPC], mybir.dt.float32r)
            paugT_sb = consts.tile([KAUG, NV], mybir.dt.float32r)
            nc.sync.dma_start(out=xaugT_sb[:], in_=io["xaugT"][:])
            nc.sync.dma_start(out=paugT_sb[:], in_=io["paugT"][:])

            if outer_loop is None:
                _emit_body(nc, io, singles, dists, psums, apsums,
                           xaugT_sb, paugT_sb)
            else:
                with tc.For_i(0, outer_loop, 1):
                    for _ in range(inner_unroll):
                        _emit_body(nc, io, singles, dists, psums, apsums,
                                   xaugT_sb, paugT_sb)

    nc.compile()
    return nc


def _get_program():
    if "nc" not in _CACHE:
        _CACHE["nc"] = _build_program()
    return _CACHE["nc"]


def _prep_inputs(x, p):
    x = np.ascontiguousarray(np.asarray(x, dtype=np.float32))
    p = np.ascontiguousarray(np.asarray(p, dtype=np.float32))
    xsq = np.einsum("id,id->i", x.astype(np.float64), x.astype(np.float64))
    psq = np.einsum("jd,jd->j", p.astype(np.float64), p.astype(np.float64))
    xaugT = np.empty((KAUG, BATCH), np.float32)
    xaugT[:D] = x.T
    xaugT[D] = xsq.astype(np.float32)
    xaugT[D + 1] = 1.0
    paugT = np.empty((KAUG, NV), np.float32)
    paugT[:D] = -2.0 * p.T
    paugT[D] = 1.0
    paugT[D + 1] = psq.astype(np.float32)
    in_maps = []
    for c in range(NCORES):
        in_maps.append({
            "xaugT": np.ascontiguousarray(xaugT[:, c * BPC:(c + 1) * BPC]),
            "paugT": paugT,
        })
    return in_maps


def _run(x, p, trace=False, nc=None, **kwargs):
    if nc is None:
        nc = _get_program()
    in_maps = _prep_inputs(x, p)
    return run_bass_kernel_spmd(
        nc, in_maps, core_ids=list(range(NCORES)), trace=trace, **kwargs
    )


def _assemble(results):
    dist_full = np.empty((BATCH, NV), np.float32)
    colfins = np.empty((NCORES, P, NJC), np.float32)
    rowmins = np.empty((NCORES, P, NTILES), np.float32)
    for c in range(NCORES):
        r = results[c]
        dist_full[c * BPC:(c + 1) * BPC] = r["dist"]
        colfins[c] = r["colfin"]
        rowmins[c] = r["rowmin"]
    # cross-core all-reduce-min over batch (D2 domain), then codebook mean
    r1 = np.float32(np.mean(np.sqrt(colfins.min(axis=0).astype(np.float64))))
    # per-sample min is already complete locally (D2 domain); batch mean
    r2 = np.float32(np.mean(np.sqrt(rowmins.astype(np.float64))))
    return dist_full, r1, r2


def kernel(x, trainable_p):
    res = _run(x, trainable_p)
    return _assemble(res.results)
